# revision 13
# baseline (speedup 1.0000x reference)
"""Trainium2 Bass kernel for per-position head-mixing attention.

Math (per position p): Qh,Kh,Vh = reshape(q/k/v[p], [16, 64]);
L = Qh @ Kh.T / 8; W = softmax(L, axis=-1); out[p] = W @ Vh.

Strategy:
  * Pure data parallel over 8 cores (4096 positions each).
  * Host pre-transposes q,k to [d, ...] layout (exact, host-side) and casts to
    bf16 so every device DMA is large and contiguous.
  * Device: per group of 8 positions, one 73x128x128 matmul computes all
    16x16 logit blocks; 9 extra contraction rows add -C to off-diagonal
    (cross-position) entries so exp() zeroes them - no masking op needed.
    The 9 mask rows live at partitions 64..72 of a persistent double buffer,
    written once; per-tile DMAs refresh only partitions 0..63.
  * exp on ScalarE (scale=1/8 fused), batched over 8 groups.
  * Second matmul W' @ [V | 1] gives numerator and softmax denominator in one
    pass; VectorE reciprocal+multiply normalizes (fp16 out).
  * DMA routing: big loads on gpsimd/SWDGE (16-engine spray), output store on
    the sync HWDGE ring so stores never block loads.
"""

import sys

if "/opt/trn_rl_repo" not in sys.path:
    sys.path.insert(0, "/opt/trn_rl_repo")

from contextlib import ExitStack

import ml_dtypes
import numpy as np

import concourse.bass as bass
from concourse import bacc, mybir, tile
from concourse.bass_utils import run_bass_kernel_spmd

BF16 = mybir.dt.bfloat16
F16 = mybir.dt.float16
F32 = mybir.dt.float32
NPBF16 = ml_dtypes.bfloat16

N_CORES = 8
S_TOT = 4 * 8192          # flattened (batch, seq) positions
H, D = 16, 64             # heads, key size
N_PC = S_TOT // N_CORES   # positions per core
T = 512                   # positions per on-chip tile
G = T // 8                # 8-position groups per tile
NT = N_PC // T            # tiles per core
B = 8                     # groups per psum/exp batch
NB = G // B
C_MASK = 400.0            # off-diagonal logit penalty (exact in bf16)
SCALE = 0.125             # 1/sqrt(64)

_CACHE = {}


def _build_program(nt: int, n_cores: int):
    nc = bacc.Bacc(
        "TRN2", target_bir_lowering=False, debug=False, num_devices=n_cores
    )
    # combined q|k data, d on partitions: [64, nt, {q,k}, G, H, 8]
    qk = nc.dram_tensor("qk", [64, nt, 2, G, H, 8], BF16, kind="ExternalInput").ap()
    mk = nc.dram_tensor("mk", [9, 2, G, H, 8], BF16, kind="ExternalInput").ap()
    vr = nc.dram_tensor("vr", [H, 8, nt, G, 65], BF16, kind="ExternalInput").ap()
    out = nc.dram_tensor("o", [H, 8, nt, G, D], F16, kind="ExternalOutput").ap()

    QKW = 2 * G * 128  # columns in a combined q|k tile (q first, then k)

    with tile.TileContext(nc) as tc, ExitStack() as ctx:
        qk_pool = ctx.enter_context(tc.tile_pool(name="qk", bufs=1))
        v_pool = ctx.enter_context(tc.tile_pool(name="v", bufs=4))
        o_pool = ctx.enter_context(tc.tile_pool(name="o", bufs=3))
        w_pool = ctx.enter_context(tc.tile_pool(name="w", bufs=3))
        r_pool = ctx.enter_context(tc.tile_pool(name="r", bufs=3))
        p1_pool = ctx.enter_context(tc.tile_pool(name="p1", bufs=2, space="PSUM"))
        p2_pool = ctx.enter_context(tc.tile_pool(name="p2", bufs=2, space="PSUM"))

        # persistent double buffer for q|k, mask rows written once
        qk_bufs = [
            qk_pool.tile([73, QKW], BF16, tag="qk0", name="qkbuf0"),
            qk_pool.tile([73, QKW], BF16, tag="qk1", name="qkbuf1"),
            qk_pool.tile([73, QKW], BF16, tag="qk2", name="qkbuf2"),
        ]
        mflat = mk.rearrange("p a b c d -> p (a b c d)")
        for buf in qk_bufs:
            nc.gpsimd.dma_start(buf[64:73, :], mflat)

        def load_qk(i, split=1):
            # split > 1 loads group-ranges in separate DMAs so the first
            # batches can start before the whole tile has landed (tile 0)
            src = qk[:, i].rearrange("p a b c d -> p a (b c d)")
            gw = G * 128
            for s in range(split):
                lo, hi = s * (gw // split), (s + 1) * (gw // split)
                for side in range(2):
                    nc.gpsimd.dma_start(
                        qk_bufs[i % 3][0:64, side * gw + lo : side * gw + hi],
                        src[:, side, lo:hi],
                    )

        def load_v(i):
            v_t = v_pool.tile([128, G * 65], BF16)
            nc.gpsimd.dma_start(v_t[:], vr[:, :, i].rearrange("k p g e -> (k p) (g e)"))
            return v_t

        v_tiles = {0: load_v(0)}
        load_qk(0, split=4)
        if nt > 1:
            load_qk(1)
            v_tiles[1] = load_v(1)

        for i in range(nt):
            # prefetch two tiles ahead so the (even-numbered) SDMA engines
            # that carry the 64-partition qk stream never starve
            if i + 2 < nt:
                load_qk(i + 2)
                v_tiles[i + 2] = load_v(i + 2)
            qk_t = qk_bufs[i % 3]
            v_t = v_tiles.pop(i)
            o_t = o_pool.tile([128, G * 64], F16)

            stash = None
            for b in range(NB + 1):
                if b < NB:
                    p1 = p1_pool.tile([128, B * 128], F32)
                    for j in range(B):
                        g = b * B + j
                        nc.tensor.matmul(
                            p1[:, j * 128 : (j + 1) * 128],
                            lhsT=qk_t[:, G * 128 + g * 128 : G * 128 + (g + 1) * 128],
                            rhs=qk_t[:, g * 128 : (g + 1) * 128],
                            start=True,
                            stop=True,
                        )
                    w = w_pool.tile([128, B * 128], BF16)
                    nc.scalar.activation(
                        w[:], p1[:], mybir.ActivationFunctionType.Exp, scale=SCALE
                    )
                else:
                    w = None
                if stash is not None:
                    wp, bp = stash
                    p2 = p2_pool.tile([128, B * 128], F32)
                    for j in range(B):
                        g = bp * B + j
                        nc.tensor.matmul(
                            p2[:, j * 128 : j * 128 + 65],
                            lhsT=wp[:, j * 128 : (j + 1) * 128],
                            rhs=v_t[:, g * 65 : (g + 1) * 65],
                            start=True,
                            stop=True,
                        )
                    r = r_pool.tile([128, B], F32)
                    p2v = p2[:].rearrange("p (g c) -> p g c", c=128)
                    nc.vector.reciprocal(r[:], p2v[:, :, 64])
                    rb = r[:].unsqueeze(2).broadcast_to([128, B, 64])
                    ov = o_t[:, bp * B * 64 : (bp + 1) * B * 64].rearrange(
                        "p (g c) -> p g c", c=64
                    )
                    nc.vector.tensor_tensor(
                        ov, p2v[:, :, 0:64], rb, op=mybir.AluOpType.mult
                    )
                stash = (w, b) if w is not None else None
            oflat = out[:, :, i].rearrange("k p g e -> (k p) (g e)")
            half = G * 64 // 2
            nc.sync.dma_start(oflat[:, :half], o_t[:, :half])
            nc.sync.dma_start(oflat[:, half:], o_t[:, half:])

    nc.compile()
    return nc


def _prep_qk(qslab: np.ndarray, kslab: np.ndarray, nt: int) -> np.ndarray:
    """Two [nt*T, 1024] fp32 slabs -> [64, nt, 2, G, H, 8] bf16 (d-major)."""
    full = np.empty((64, nt, 2, G, H, 8), dtype=NPBF16)
    for s, slab in enumerate((qslab, kslab)):
        a = slab.reshape(nt, G, 8, H, D)            # [i, g, p, h, d]
        full[:, :, s] = a.transpose(4, 0, 1, 3, 2).astype(NPBF16)
    return full


def _mask_const() -> np.ndarray:
    """[9, 2, G, H, 8] bf16: rows 64..72 of the combined q|k tile."""
    m = np.zeros((9, 2, G, H, 8), dtype=NPBF16)
    m[0, 0] = NPBF16(1.0)        # q side, row 64: ones
    m[0, 1] = NPBF16(-C_MASK)    # k side, row 64: -C
    for j in range(8):
        m[1 + j, 0, :, :, j] = NPBF16(1.0)     # q side: delta(p, j)
        m[1 + j, 1, :, :, j] = NPBF16(C_MASK)  # k side: C * delta(p, j)
    return m


def _prep_v(slab: np.ndarray, nt: int) -> np.ndarray:
    """[nt*T, 1024] fp32 -> [H, 8, nt, G, 65] bf16 with ones column."""
    a = slab.reshape(nt, G, 8, H, D)
    full = np.empty((H, 8, nt, G, 65), dtype=NPBF16)
    full[..., :64] = a.transpose(3, 2, 0, 1, 4).astype(NPBF16)
    full[..., 64] = NPBF16(1.0)
    return full


def kernel(q: np.ndarray, k: np.ndarray, v: np.ndarray) -> np.ndarray:
    bshape = q.shape
    qf = np.ascontiguousarray(np.asarray(q, dtype=np.float32)).reshape(S_TOT, H * D)
    kf = np.ascontiguousarray(np.asarray(k, dtype=np.float32)).reshape(S_TOT, H * D)
    vf = np.ascontiguousarray(np.asarray(v, dtype=np.float32)).reshape(S_TOT, H * D)

    key = (NT, N_CORES)
    if key not in _CACHE:
        _CACHE[key] = _build_program(*key)
    nc = _CACHE[key]

    mk = _mask_const()
    in_maps = []
    for c in range(N_CORES):
        s0, s1 = c * N_PC, (c + 1) * N_PC
        in_maps.append(
            {
                "qk": _prep_qk(qf[s0:s1], kf[s0:s1], NT),
                "mk": mk,
                "vr": _prep_v(vf[s0:s1], NT),
            }
        )

    res = run_bass_kernel_spmd(nc, in_maps, core_ids=list(range(N_CORES)))

    out = np.empty((S_TOT, H * D), dtype=np.float32)
    for c in range(N_CORES):
        o = res.results[c]["o"]  # [H, 8, NT, G, D] fp16
        out[c * N_PC : (c + 1) * N_PC] = (
            o.transpose(2, 3, 1, 0, 4).reshape(N_PC, H * D).astype(np.float32)
        )
    return out.reshape(bshape)


# revision 17
# speedup vs baseline: 1.0363x; 1.0363x over previous
"""Trainium2 Bass kernel for per-position head-mixing attention.

Math (per position p): Qh,Kh,Vh = reshape(q/k/v[p], [16, 64]);
L = Qh @ Kh.T / 8; W = softmax(L, axis=-1); out[p] = W @ Vh.

Strategy:
  * Pure data parallel over 8 cores (4096 positions each).
  * Host pre-transposes q,k to [d, ...] layout (exact, host-side) and casts to
    bf16 so every device DMA is large and contiguous.
  * Device: per group of 8 positions, one 73x128x128 matmul computes all
    16x16 logit blocks; 9 extra contraction rows add -C to off-diagonal
    (cross-position) entries so exp() zeroes them - no masking op needed.
    The 9 mask rows live at partitions 64..72 of a persistent double buffer,
    written once; per-tile DMAs refresh only partitions 0..63.
  * exp on ScalarE (scale=1/8 fused), batched over 8 groups.
  * Second matmul W' @ [V | 1] gives numerator and softmax denominator in one
    pass; VectorE reciprocal+multiply normalizes (fp16 out).
  * DMA routing: big loads on gpsimd/SWDGE (16-engine spray), output store on
    the sync HWDGE ring so stores never block loads.
"""

import sys

if "/opt/trn_rl_repo" not in sys.path:
    sys.path.insert(0, "/opt/trn_rl_repo")

from contextlib import ExitStack

import ml_dtypes
import numpy as np

import concourse.bass as bass
from concourse import bacc, mybir, tile
from concourse.bass_utils import run_bass_kernel_spmd

BF16 = mybir.dt.bfloat16
F16 = mybir.dt.float16
F32 = mybir.dt.float32
NPBF16 = ml_dtypes.bfloat16

N_CORES = 8
S_TOT = 4 * 8192          # flattened (batch, seq) positions
H, D = 16, 64             # heads, key size
N_PC = S_TOT // N_CORES   # positions per core
T = 512                   # positions per on-chip tile
G = T // 8                # 8-position groups per tile
NT = N_PC // T            # tiles per core
B = 8                     # groups per psum/exp batch
NB = G // B
C_MASK = 400.0            # off-diagonal logit penalty (exact in bf16)
SCALE = 0.125             # 1/sqrt(64)

_CACHE = {}


def _build_program(nt: int, n_cores: int):
    nc = bacc.Bacc(
        "TRN2", target_bir_lowering=False, debug=False, num_devices=n_cores
    )
    # combined q|k data, d on partitions: [64, nt, {q,k}, G, H, 8]
    qk = nc.dram_tensor("qk", [64, nt, 2, G, H, 8], BF16, kind="ExternalInput").ap()
    mk = nc.dram_tensor("mk", [9, 2, G, H, 8], BF16, kind="ExternalInput").ap()
    vr = nc.dram_tensor("vr", [H, 8, nt, G, 65], BF16, kind="ExternalInput").ap()
    out = nc.dram_tensor("o", [H, 8, nt, G, D], F16, kind="ExternalOutput").ap()

    QKW = 2 * G * 128  # columns in a combined q|k tile (q first, then k)

    with tile.TileContext(nc) as tc, ExitStack() as ctx:
        qk_pool = ctx.enter_context(tc.tile_pool(name="qk", bufs=1))
        v_pool = ctx.enter_context(tc.tile_pool(name="v", bufs=3))
        o_pool = ctx.enter_context(tc.tile_pool(name="o", bufs=3))
        w_pool = ctx.enter_context(tc.tile_pool(name="w", bufs=3))
        r_pool = ctx.enter_context(tc.tile_pool(name="r", bufs=3))
        p1_pool = ctx.enter_context(tc.tile_pool(name="p1", bufs=2, space="PSUM"))
        p2_pool = ctx.enter_context(tc.tile_pool(name="p2", bufs=2, space="PSUM"))

        # persistent double buffer for q|k, mask rows written once
        qk_bufs = [
            qk_pool.tile([73, QKW], BF16, tag="qk0", name="qkbuf0"),
            qk_pool.tile([73, QKW], BF16, tag="qk1", name="qkbuf1"),
        ]
        mflat = mk.rearrange("p a b c d -> p (a b c d)")
        for buf in qk_bufs:
            nc.gpsimd.dma_start(buf[64:73, :], mflat)

        def load_qk(i, split=1):
            # split > 1 loads group-ranges in separate DMAs so the first
            # batches can start before the whole tile has landed (tile 0)
            src = qk[:, i].rearrange("p a b c d -> p a (b c d)")
            gw = G * 128
            for s in range(split):
                lo, hi = s * (gw // split), (s + 1) * (gw // split)
                for side in range(2):
                    nc.gpsimd.dma_start(
                        qk_bufs[i % 2][0:64, side * gw + lo : side * gw + hi],
                        src[:, side, lo:hi],
                    )

        def load_v(i):
            v_t = v_pool.tile([128, G * 65], BF16)
            nc.gpsimd.dma_start(v_t[:], vr[:, :, i].rearrange("k p g e -> (k p) (g e)"))
            return v_t

        v_tiles = {0: load_v(0)}
        load_qk(0, split=4)

        for i in range(nt):
            # prefetch next tile's inputs ahead of this tile's compute so the
            # gpsimd DMA queue never sits behind compute-dependent work
            if i + 1 < nt:
                load_qk(i + 1)
                v_tiles[i + 1] = load_v(i + 1)
            qk_t = qk_bufs[i % 2]
            v_t = v_tiles.pop(i)
            o_t = o_pool.tile([128, G * 64], F16)

            stash = None
            for b in range(NB + 1):
                if b < NB:
                    p1 = p1_pool.tile([128, B * 128], F32)
                    for j in range(B):
                        g = b * B + j
                        nc.tensor.matmul(
                            p1[:, j * 128 : (j + 1) * 128],
                            lhsT=qk_t[:, G * 128 + g * 128 : G * 128 + (g + 1) * 128],
                            rhs=qk_t[:, g * 128 : (g + 1) * 128],
                            start=True,
                            stop=True,
                        )
                    w = w_pool.tile([128, B * 128], BF16)
                    nc.scalar.activation(
                        w[:], p1[:], mybir.ActivationFunctionType.Exp, scale=SCALE
                    )
                else:
                    w = None
                if stash is not None:
                    wp, bp = stash
                    p2 = p2_pool.tile([128, B * 128], F32)
                    for j in range(B):
                        g = bp * B + j
                        nc.tensor.matmul(
                            p2[:, j * 128 : j * 128 + 65],
                            lhsT=wp[:, j * 128 : (j + 1) * 128],
                            rhs=v_t[:, g * 65 : (g + 1) * 65],
                            start=True,
                            stop=True,
                        )
                    r = r_pool.tile([128, B], F32)
                    p2v = p2[:].rearrange("p (g c) -> p g c", c=128)
                    nc.vector.reciprocal(r[:], p2v[:, :, 64])
                    rb = r[:].unsqueeze(2).broadcast_to([128, B, 64])
                    ov = o_t[:, bp * B * 64 : (bp + 1) * B * 64].rearrange(
                        "p (g c) -> p g c", c=64
                    )
                    nc.vector.tensor_tensor(
                        ov, p2v[:, :, 0:64], rb, op=mybir.AluOpType.mult
                    )
                stash = (w, b) if w is not None else None
            oflat = out[:, :, i].rearrange("k p g e -> (k p) (g e)")
            half = G * 64 // 2
            nc.sync.dma_start(oflat[:, :half], o_t[:, :half])
            nc.sync.dma_start(oflat[:, half:], o_t[:, half:])

    nc.compile()
    return nc


def _prep_qk(qslab: np.ndarray, kslab: np.ndarray, nt: int) -> np.ndarray:
    """Two [nt*T, 1024] fp32 slabs -> [64, nt, 2, G, H, 8] bf16 (d-major)."""
    full = np.empty((64, nt, 2, G, H, 8), dtype=NPBF16)
    for s, slab in enumerate((qslab, kslab)):
        a = slab.reshape(nt, G, 8, H, D)            # [i, g, p, h, d]
        full[:, :, s] = a.transpose(4, 0, 1, 3, 2).astype(NPBF16)
    return full


def _mask_const() -> np.ndarray:
    """[9, 2, G, H, 8] bf16: rows 64..72 of the combined q|k tile."""
    m = np.zeros((9, 2, G, H, 8), dtype=NPBF16)
    m[0, 0] = NPBF16(1.0)        # q side, row 64: ones
    m[0, 1] = NPBF16(-C_MASK)    # k side, row 64: -C
    for j in range(8):
        m[1 + j, 0, :, :, j] = NPBF16(1.0)     # q side: delta(p, j)
        m[1 + j, 1, :, :, j] = NPBF16(C_MASK)  # k side: C * delta(p, j)
    return m


def _prep_v(slab: np.ndarray, nt: int) -> np.ndarray:
    """[nt*T, 1024] fp32 -> [H, 8, nt, G, 65] bf16 with ones column."""
    a = slab.reshape(nt, G, 8, H, D)
    full = np.empty((H, 8, nt, G, 65), dtype=NPBF16)
    full[..., :64] = a.transpose(3, 2, 0, 1, 4).astype(NPBF16)
    full[..., 64] = NPBF16(1.0)
    return full


def kernel(q: np.ndarray, k: np.ndarray, v: np.ndarray) -> np.ndarray:
    bshape = q.shape
    qf = np.ascontiguousarray(np.asarray(q, dtype=np.float32)).reshape(S_TOT, H * D)
    kf = np.ascontiguousarray(np.asarray(k, dtype=np.float32)).reshape(S_TOT, H * D)
    vf = np.ascontiguousarray(np.asarray(v, dtype=np.float32)).reshape(S_TOT, H * D)

    key = (NT, N_CORES)
    if key not in _CACHE:
        _CACHE[key] = _build_program(*key)
    nc = _CACHE[key]

    mk = _mask_const()
    in_maps = []
    for c in range(N_CORES):
        s0, s1 = c * N_PC, (c + 1) * N_PC
        in_maps.append(
            {
                "qk": _prep_qk(qf[s0:s1], kf[s0:s1], NT),
                "mk": mk,
                "vr": _prep_v(vf[s0:s1], NT),
            }
        )

    res = run_bass_kernel_spmd(nc, in_maps, core_ids=list(range(N_CORES)))

    out = np.empty((S_TOT, H * D), dtype=np.float32)
    for c in range(N_CORES):
        o = res.results[c]["o"]  # [H, 8, NT, G, D] fp16
        out[c * N_PC : (c + 1) * N_PC] = (
            o.transpose(2, 3, 1, 0, 4).reshape(N_PC, H * D).astype(np.float32)
        )
    return out.reshape(bshape)
